# revision 10
# baseline (speedup 1.0000x reference)
"""Trainium2 Bass kernel for AbstractMaxpool2D.

Computes, for inputs x_center/x_abs/x_true of shape [128, 512, 512] f32:
  out_c    = maxpool2x2(x_center)
  out_min  = maxpool2x2(x_center - x_abs)
  out_max  = maxpool2x2(x_center + x_abs)
  out_true = maxpool2x2(x_true)
each [128, 256, 256] f32.  (The reference's relu-chain is exactly a 2x2
window max up to fp32 rounding; we compute the max directly.)

Sharding: channel dim C=128 split across 8 NeuronCores (16 channels each).
Per core the stream is a flat [8192, 512] row-major image; each tile is
[128 partitions x 4096] = 8 consecutive rows per partition, so both the
vertical (row-pair) and horizontal (col-pair) max reductions are
per-partition DVE ops and every DMA is a fully contiguous 2MB transfer.
"""

import numpy as np

try:
    import concourse.bass as bass
except ImportError:  # pragma: no cover - fallback for fresh grading dir
    import sys

    sys.path.insert(0, "/opt/trn_rl_repo")
    import concourse.bass as bass

import concourse.tile as tile
from concourse import mybir
from concourse.bass_utils import run_bass_kernel_spmd

F32 = mybir.dt.float32

N_CORES = 8
C, H, W = 128, 512, 512
CPC = C // N_CORES  # channels per core
P = 128  # SBUF partitions
ROWS_PER_PART = 8  # input image rows held by one partition per tile
TILE_F = ROWS_PER_PART * W  # 4096 floats per partition per input tile
OUT_F = (ROWS_PER_PART // 2) * (W // 2)  # 1024 floats per partition per out tile
N_ITERS = (CPC * H) // (P * ROWS_PER_PART)  # 8

IN_STREAMS = ("x_center", "x_abs", "x_true")
OUT_STREAMS = ("out_c", "out_min", "out_max", "out_true")

_CACHE = {}


def _split_excess_waits(nc):
    """Each 64B ISA instruction has ONE sync-wait slot (EventSemaphore: 2).

    Tile's sem assignment can attach several waits to one instruction;
    walrus then fails with 'Too many sync wait commands'.  Move the excess
    onto standalone EventSemaphore (wait-only) instructions placed just
    before, on the same engine — semantically identical, sequencer executes
    them in order.
    """
    n = 0
    for func in nc.m.functions:
        for blk in func.blocks:
            new_insts = []
            for inst in blk.instructions:
                si = inst.sync_info
                cap = 2 if isinstance(inst, mybir.InstEventSemaphore) else 1
                if si is not None and len(si.on_wait) > cap:
                    waits = list(si.on_wait)
                    keep, extra = waits[-cap:], waits[:-cap]
                    for w in extra:
                        n += 1
                        nop = mybir.InstEventSemaphore(
                            name=f"I-waitsplit-{n}", ins=[], outs=[]
                        )
                        nop.engine = inst.engine
                        nop.sync_info = mybir.SyncInfo(on_wait=[w], on_update=[])
                        new_insts.append(nop)
                    inst.sync_info = mybir.SyncInfo(
                        on_wait=keep, on_update=list(si.on_update)
                    )
                new_insts.append(inst)
            blk.instructions = new_insts
    return n


MM_F = 512  # fp32 matmul moving-operand max free dim
PS_F = 1024  # psum tile free dim (2 banks); 2 matmul chunks per tile
# Engine balance: PE fp32 identity-matmul costs ~1.74us per 512 cols
# (2 logical mm -> 4 HW passes), DVE tensor_sub ~0.6us per 512 cols.
# With DVE also running the 4 pooling chains (~13.6us/iter), putting all
# of sum + 1024 cols of diff on PE equalizes PE/DVE at ~17us/iter, both
# under the ~20.5us/iter DMA budget.
N_DIFF_PE = 1024  # first 1024 cols of (c-a) computed on PE, rest on DVE


def _build_nc():
    nc = bass.Bass(trn_type="TRN2")
    ins = {
        nm: nc.dram_tensor(nm, [N_ITERS, P, TILE_F], F32, kind="ExternalInput")
        for nm in IN_STREAMS
    }
    # idents[0] = I, idents[1] = -I (fp32 matmuls self-load weights, so
    # alternating weights costs nothing; identity matmul is bit-exact).
    ident_in = nc.dram_tensor("idents", [2, P, P], F32, kind="ExternalInput")
    outs = {
        nm: nc.dram_tensor(nm, [N_ITERS, P, OUT_F], F32, kind="ExternalOutput")
        for nm in OUT_STREAMS
    }

    with tile.TileContext(nc) as tc:
        with tc.tile_pool(name="const", bufs=1) as cpool, tc.tile_pool(
            name="io_in", bufs=2
        ) as inpool, tc.tile_pool(name="scratch", bufs=3) as spool, tc.tile_pool(
            name="vmpool", bufs=2
        ) as vmpool, tc.tile_pool(name="io_out", bufs=2) as opool, tc.tile_pool(
            name="psum", bufs=4, space="PSUM"
        ) as pspool:
            eye = cpool.tile([P, P], F32, name="eye")
            nc.sync.dma_start(eye, ident_in[0])
            neye = cpool.tile([P, P], F32, name="neye")
            nc.sync.dma_start(neye, ident_in[1])

            def pool22(src, oname, i):
                # src: AP [P, TILE_F]; rows r=0..7 per partition at offset r*W.
                # Vertical max of row pairs (2q, 2q+1) -> vm[q*W + w].
                vm = vmpool.tile([P, TILE_F // 2], F32, name="vm", tag="vm")
                s4 = src.rearrange("p (q two w) -> p q two w", two=2, w=W)
                v3 = vm.rearrange("p (q w) -> p q w", w=W)
                nc.vector.tensor_max(v3, s4[:, :, 0, :], s4[:, :, 1, :])
                # Horizontal max of col pairs -> o[q*(W//2) + w'].
                o = opool.tile([P, OUT_F], F32, name=oname, tag=oname)
                vp = vm.rearrange("p (k two) -> p k two", two=2)
                nc.vector.tensor_max(o, vp[:, :, 0], vp[:, :, 1])
                nc.scalar.dma_start(outs[oname][i], o)

            def pe_combine(dst, c_t, a_t, a_eye, lo, hi):
                # dst[:, lo:hi] = c +/- a via identity matmuls into PSUM,
                # copied to SBUF by the (otherwise idle) scalar engine.
                for p0 in range(lo, hi, PS_F):
                    ps = pspool.tile([P, PS_F], F32, name="ps", tag="ps")
                    for k0 in range(0, PS_F, MM_F):
                        sl = slice(p0 + k0, p0 + k0 + MM_F)
                        psl = slice(k0, k0 + MM_F)
                        nc.tensor.matmul(
                            ps[:, psl], eye, c_t[:, sl], start=True, stop=False
                        )
                        nc.tensor.matmul(
                            ps[:, psl], a_eye, a_t[:, sl], start=False, stop=True
                        )
                    nc.scalar.copy(dst[:, p0 : p0 + PS_F], ps)

            for i in range(N_ITERS):
                # Load order t, c, a: t's pooling runs first on DVE so its
                # slot frees earliest; keeps the FIFO load queue flowing.
                t_t = inpool.tile([P, TILE_F], F32, name="t_t", tag="t_t")
                nc.sync.dma_start(t_t, ins["x_true"][i])
                c_t = inpool.tile([P, TILE_F], F32, name="c_t", tag="c_t")
                nc.sync.dma_start(c_t, ins["x_center"][i])
                a_t = inpool.tile([P, TILE_F], F32, name="a_t", tag="a_t")
                nc.sync.dma_start(a_t, ins["x_abs"][i])

                # sum = c + a entirely on PE; diff = c - a split PE/DVE.
                s = spool.tile([P, TILE_F], F32, name="s", tag="sd")
                pe_combine(s, c_t, a_t, eye, 0, TILE_F)
                d = spool.tile([P, TILE_F], F32, name="d", tag="sd")
                pe_combine(d, c_t, a_t, neye, 0, N_DIFF_PE)

                pool22(t_t, "out_true", i)
                pool22(c_t, "out_c", i)
                nc.vector.tensor_sub(
                    d[:, N_DIFF_PE:], c_t[:, N_DIFF_PE:], a_t[:, N_DIFF_PE:]
                )
                pool22(d, "out_min", i)
                pool22(s, "out_max", i)

    _split_excess_waits(nc)
    return nc


def _get_nc():
    if "nc" not in _CACHE:
        _CACHE["nc"] = _build_nc()
    return _CACHE["nc"]


def _shard_inputs(inputs):
    eye = np.eye(P, dtype=np.float32)
    idents = np.stack([eye, -eye])
    in_maps = []
    for k in range(N_CORES):
        sl = slice(k * CPC, (k + 1) * CPC)
        m = {
            nm: np.ascontiguousarray(inputs[nm][sl], dtype=np.float32).reshape(
                N_ITERS, P, TILE_F
            )
            for nm in IN_STREAMS
        }
        m["idents"] = idents
        in_maps.append(m)
    return in_maps


def _gather_outputs(results):
    outs = []
    for nm in OUT_STREAMS:
        outs.append(
            np.concatenate(
                [results[k][nm].reshape(CPC, H // 2, W // 2) for k in range(N_CORES)],
                axis=0,
            )
        )
    return tuple(outs)


def _run(inputs, **kwargs):
    nc = _get_nc()
    in_maps = _shard_inputs(inputs)
    return run_bass_kernel_spmd(nc, in_maps, core_ids=list(range(N_CORES)), **kwargs)


def kernel(x_center, x_abs, x_true):
    res = _run({"x_center": x_center, "x_abs": x_abs, "x_true": x_true})
    return _gather_outputs(res.results)


# revision 11
# speedup vs baseline: 1.0068x; 1.0068x over previous
"""Trainium2 Bass kernel for AbstractMaxpool2D.

Computes, for inputs x_center/x_abs/x_true of shape [128, 512, 512] f32:
  out_c    = maxpool2x2(x_center)
  out_min  = maxpool2x2(x_center - x_abs)
  out_max  = maxpool2x2(x_center + x_abs)
  out_true = maxpool2x2(x_true)
each [128, 256, 256] f32.  (The reference's relu-chain is exactly a 2x2
window max up to fp32 rounding; we compute the max directly.)

Sharding: channel dim C=128 split across 8 NeuronCores (16 channels each).
Per core the stream is a flat [8192, 512] row-major image; each tile is
[128 partitions x 4096] = 8 consecutive rows per partition, so both the
vertical (row-pair) and horizontal (col-pair) max reductions are
per-partition DVE ops and every DMA is a fully contiguous 2MB transfer.
"""

import numpy as np

try:
    import concourse.bass as bass
except ImportError:  # pragma: no cover - fallback for fresh grading dir
    import sys

    sys.path.insert(0, "/opt/trn_rl_repo")
    import concourse.bass as bass

import concourse.tile as tile
from concourse import mybir
from concourse.bass_utils import run_bass_kernel_spmd

F32 = mybir.dt.float32

N_CORES = 8
C, H, W = 128, 512, 512
CPC = C // N_CORES  # channels per core
P = 128  # SBUF partitions
ROWS_PER_PART = 8  # input image rows held by one partition per tile
TILE_F = ROWS_PER_PART * W  # 4096 floats per partition per input tile
OUT_F = (ROWS_PER_PART // 2) * (W // 2)  # 1024 floats per partition per out tile
N_ITERS = (CPC * H) // (P * ROWS_PER_PART)  # 8

IN_STREAMS = ("x_center", "x_abs", "x_true")
OUT_STREAMS = ("out_c", "out_min", "out_max", "out_true")

_CACHE = {}


def _split_excess_waits(nc):
    """Each 64B ISA instruction has ONE sync-wait slot (EventSemaphore: 2).

    Tile's sem assignment can attach several waits to one instruction;
    walrus then fails with 'Too many sync wait commands'.  Move the excess
    onto standalone EventSemaphore (wait-only) instructions placed just
    before, on the same engine — semantically identical, sequencer executes
    them in order.
    """
    n = 0
    for func in nc.m.functions:
        for blk in func.blocks:
            new_insts = []
            for inst in blk.instructions:
                si = inst.sync_info
                cap = 2 if isinstance(inst, mybir.InstEventSemaphore) else 1
                if si is not None and len(si.on_wait) > cap:
                    waits = list(si.on_wait)
                    keep, extra = waits[-cap:], waits[:-cap]
                    for w in extra:
                        n += 1
                        nop = mybir.InstEventSemaphore(
                            name=f"I-waitsplit-{n}", ins=[], outs=[]
                        )
                        nop.engine = inst.engine
                        nop.sync_info = mybir.SyncInfo(on_wait=[w], on_update=[])
                        new_insts.append(nop)
                    inst.sync_info = mybir.SyncInfo(
                        on_wait=keep, on_update=list(si.on_update)
                    )
                new_insts.append(inst)
            blk.instructions = new_insts
    return n


MM_F = 512  # fp32 matmul moving-operand max free dim
PS_F = 1024  # psum tile free dim (2 banks); 2 matmul chunks per tile
# Engine balance: PE fp32 identity-matmul costs ~1.74us per 512 cols
# (2 logical mm -> 4 HW passes), DVE tensor_sub ~0.6us per 512 cols.
# With DVE also running the 4 pooling chains (~13.6us/iter), putting all
# of sum + 1024 cols of diff on PE equalizes PE/DVE at ~17us/iter, both
# under the ~20.5us/iter DMA budget.
N_DIFF_PE = 1024  # first 1024 cols of (c-a) computed on PE, rest on DVE


def _build_nc():
    nc = bass.Bass(trn_type="TRN2", dynamic_dma_scratch_size=2048)
    ins = {
        nm: nc.dram_tensor(nm, [N_ITERS, P, TILE_F], F32, kind="ExternalInput")
        for nm in IN_STREAMS
    }
    # idents[0] = I, idents[1] = -I (fp32 matmuls self-load weights, so
    # alternating weights costs nothing; identity matmul is bit-exact).
    ident_in = nc.dram_tensor("idents", [2, P, P], F32, kind="ExternalInput")
    outs = {
        nm: nc.dram_tensor(nm, [N_ITERS, P, OUT_F], F32, kind="ExternalOutput")
        for nm in OUT_STREAMS
    }

    with tile.TileContext(nc) as tc:
        with tc.tile_pool(name="const", bufs=1) as cpool, tc.tile_pool(
            name="io_in", bufs=2
        ) as inpool, tc.tile_pool(name="scratch", bufs=3) as spool, tc.tile_pool(
            name="vmpool", bufs=2
        ) as vmpool, tc.tile_pool(name="io_out", bufs=2) as opool, tc.tile_pool(
            name="psum", bufs=4, space="PSUM"
        ) as pspool:
            eye = cpool.tile([P, P], F32, name="eye")
            nc.sync.dma_start(eye, ident_in[0])
            neye = cpool.tile([P, P], F32, name="neye")
            nc.sync.dma_start(neye, ident_in[1])

            def pool22(src, oname, i):
                # src: AP [P, TILE_F]; rows r=0..7 per partition at offset r*W.
                # Vertical max of row pairs (2q, 2q+1) -> vm[q*W + w].
                vm = vmpool.tile([P, TILE_F // 2], F32, name="vm", tag="vm")
                s4 = src.rearrange("p (q two w) -> p q two w", two=2, w=W)
                v3 = vm.rearrange("p (q w) -> p q w", w=W)
                nc.vector.tensor_max(v3, s4[:, :, 0, :], s4[:, :, 1, :])
                # Horizontal max of col pairs -> o[q*(W//2) + w'].
                o = opool.tile([P, OUT_F], F32, name=oname, tag=oname)
                vp = vm.rearrange("p (k two) -> p k two", two=2)
                nc.vector.tensor_max(o, vp[:, :, 0], vp[:, :, 1])
                nc.scalar.dma_start(outs[oname][i], o)

            def pe_combine(dst, c_t, a_t, a_eye, lo, hi):
                # dst[:, lo:hi] = c +/- a via identity matmuls into PSUM,
                # copied to SBUF by the (otherwise idle) scalar engine.
                for p0 in range(lo, hi, PS_F):
                    ps = pspool.tile([P, PS_F], F32, name="ps", tag="ps")
                    for k0 in range(0, PS_F, MM_F):
                        sl = slice(p0 + k0, p0 + k0 + MM_F)
                        psl = slice(k0, k0 + MM_F)
                        nc.tensor.matmul(
                            ps[:, psl], eye, c_t[:, sl], start=True, stop=False
                        )
                        nc.tensor.matmul(
                            ps[:, psl], a_eye, a_t[:, sl], start=False, stop=True
                        )
                    nc.scalar.copy(dst[:, p0 : p0 + PS_F], ps)

            for i in range(N_ITERS):
                c_t = inpool.tile([P, TILE_F], F32, name="c_t", tag="c_t")
                nc.sync.dma_start(c_t, ins["x_center"][i])
                a_t = inpool.tile([P, TILE_F], F32, name="a_t", tag="a_t", bufs=3)
                nc.sync.dma_start(a_t, ins["x_abs"][i])
                t_t = inpool.tile([P, TILE_F], F32, name="t_t", tag="t_t")
                nc.sync.dma_start(t_t, ins["x_true"][i])

                # sum = c + a entirely on PE; diff = c - a split PE/DVE.
                s = spool.tile([P, TILE_F], F32, name="s", tag="sd")
                pe_combine(s, c_t, a_t, eye, 0, TILE_F)
                d = spool.tile([P, TILE_F], F32, name="d", tag="sd")
                pe_combine(d, c_t, a_t, neye, 0, N_DIFF_PE)
                nc.vector.tensor_sub(
                    d[:, N_DIFF_PE:], c_t[:, N_DIFF_PE:], a_t[:, N_DIFF_PE:]
                )

                pool22(c_t, "out_c", i)
                pool22(d, "out_min", i)
                pool22(s, "out_max", i)
                pool22(t_t, "out_true", i)

    _split_excess_waits(nc)
    return nc


def _get_nc():
    if "nc" not in _CACHE:
        _CACHE["nc"] = _build_nc()
    return _CACHE["nc"]


def _shard_inputs(inputs):
    eye = np.eye(P, dtype=np.float32)
    idents = np.stack([eye, -eye])
    in_maps = []
    for k in range(N_CORES):
        sl = slice(k * CPC, (k + 1) * CPC)
        m = {
            nm: np.ascontiguousarray(inputs[nm][sl], dtype=np.float32).reshape(
                N_ITERS, P, TILE_F
            )
            for nm in IN_STREAMS
        }
        m["idents"] = idents
        in_maps.append(m)
    return in_maps


def _gather_outputs(results):
    outs = []
    for nm in OUT_STREAMS:
        outs.append(
            np.concatenate(
                [results[k][nm].reshape(CPC, H // 2, W // 2) for k in range(N_CORES)],
                axis=0,
            )
        )
    return tuple(outs)


def _run(inputs, **kwargs):
    nc = _get_nc()
    in_maps = _shard_inputs(inputs)
    return run_bass_kernel_spmd(nc, in_maps, core_ids=list(range(N_CORES)), **kwargs)


def kernel(x_center, x_abs, x_true):
    res = _run({"x_center": x_center, "x_abs": x_abs, "x_true": x_true})
    return _gather_outputs(res.results)


# revision 12
# speedup vs baseline: 1.0263x; 1.0193x over previous
"""Trainium2 Bass kernel for AbstractMaxpool2D.

Computes, for inputs x_center/x_abs/x_true of shape [128, 512, 512] f32:
  out_c    = maxpool2x2(x_center)
  out_min  = maxpool2x2(x_center - x_abs)
  out_max  = maxpool2x2(x_center + x_abs)
  out_true = maxpool2x2(x_true)
each [128, 256, 256] f32.  (The reference's relu-chain is exactly a 2x2
window max up to fp32 rounding; we compute the max directly.)

Sharding: channel dim C=128 split across 8 NeuronCores (16 channels each).
Per core the stream is a flat [8192, 512] row-major image; each tile is
[128 partitions x 4096] = 8 consecutive rows per partition, so both the
vertical (row-pair) and horizontal (col-pair) max reductions are
per-partition DVE ops and every DMA is a fully contiguous 2MB transfer.
"""

import numpy as np

try:
    import concourse.bass as bass
except ImportError:  # pragma: no cover - fallback for fresh grading dir
    import sys

    sys.path.insert(0, "/opt/trn_rl_repo")
    import concourse.bass as bass

import concourse.tile as tile
from concourse import mybir
from concourse.bass_utils import run_bass_kernel_spmd

F32 = mybir.dt.float32

N_CORES = 8
C, H, W = 128, 512, 512
CPC = C // N_CORES  # channels per core
P = 128  # SBUF partitions
ROWS_PER_PART = 8  # input image rows held by one partition per tile
TILE_F = ROWS_PER_PART * W  # 4096 floats per partition per input tile
OUT_F = (ROWS_PER_PART // 2) * (W // 2)  # 1024 floats per partition per out tile
N_ITERS = (CPC * H) // (P * ROWS_PER_PART)  # 8

IN_STREAMS = ("x_center", "x_abs", "x_true")
OUT_STREAMS = ("out_c", "out_min", "out_max", "out_true")

_CACHE = {}


def _split_excess_waits(nc):
    """Each 64B ISA instruction has ONE sync-wait slot (EventSemaphore: 2).

    Tile's sem assignment can attach several waits to one instruction;
    walrus then fails with 'Too many sync wait commands'.  Move the excess
    onto standalone EventSemaphore (wait-only) instructions placed just
    before, on the same engine — semantically identical, sequencer executes
    them in order.
    """
    n = 0
    for func in nc.m.functions:
        for blk in func.blocks:
            new_insts = []
            for inst in blk.instructions:
                si = inst.sync_info
                cap = 2 if isinstance(inst, mybir.InstEventSemaphore) else 1
                if si is not None and len(si.on_wait) > cap:
                    waits = list(si.on_wait)
                    keep, extra = waits[-cap:], waits[:-cap]
                    for w in extra:
                        n += 1
                        nop = mybir.InstEventSemaphore(
                            name=f"I-waitsplit-{n}", ins=[], outs=[]
                        )
                        nop.engine = inst.engine
                        nop.sync_info = mybir.SyncInfo(on_wait=[w], on_update=[])
                        new_insts.append(nop)
                    inst.sync_info = mybir.SyncInfo(
                        on_wait=keep, on_update=list(si.on_update)
                    )
                new_insts.append(inst)
            blk.instructions = new_insts
    return n


MM_F = 512  # fp32 matmul moving-operand max free dim
PS_F = 1024  # psum tile free dim (2 banks); 2 matmul chunks per tile
# Engine balance: PE fp32 identity-matmul costs ~1.74us per 512 cols
# (2 logical mm -> 4 HW passes), DVE tensor_sub ~0.6us per 512 cols.
# With DVE also running the 4 pooling chains (~13.6us/iter), putting all
# of sum + 1024 cols of diff on PE equalizes PE/DVE at ~17us/iter, both
# under the ~20.5us/iter DMA budget.
N_DIFF_PE = 1024  # first 1024 cols of (c-a) computed on PE, rest on DVE


def _build_nc():
    nc = bass.Bass(trn_type="TRN2", dynamic_dma_scratch_size=2048)
    ins = {
        nm: nc.dram_tensor(nm, [N_ITERS, P, TILE_F], F32, kind="ExternalInput")
        for nm in IN_STREAMS
    }
    # idents[0] = I, idents[1] = -I (fp32 matmuls self-load weights, so
    # alternating weights costs nothing; identity matmul is bit-exact).
    ident_in = nc.dram_tensor("idents", [2, P, P], F32, kind="ExternalInput")
    outs = {
        nm: nc.dram_tensor(nm, [N_ITERS, P, OUT_F], F32, kind="ExternalOutput")
        for nm in OUT_STREAMS
    }

    with tile.TileContext(nc) as tc:
        with tc.tile_pool(name="const", bufs=1) as cpool, tc.tile_pool(
            name="io_in", bufs=2
        ) as inpool, tc.tile_pool(name="scratch", bufs=3) as spool, tc.tile_pool(
            name="vmpool", bufs=2
        ) as vmpool, tc.tile_pool(name="io_out", bufs=2) as opool, tc.tile_pool(
            name="psum", bufs=4, space="PSUM"
        ) as pspool:
            eye = cpool.tile([P, P], F32, name="eye")
            nc.sync.dma_start(eye, ident_in[0])
            neye = cpool.tile([P, P], F32, name="neye")
            nc.sync.dma_start(neye, ident_in[1])

            def pool22(src, oname, i):
                # src: AP [P, TILE_F]; rows r=0..7 per partition at offset r*W.
                # Vertical max of row pairs (2q, 2q+1) -> vm[q*W + w].
                vm = vmpool.tile([P, TILE_F // 2], F32, name="vm", tag="vm")
                s4 = src.rearrange("p (q two w) -> p q two w", two=2, w=W)
                v3 = vm.rearrange("p (q w) -> p q w", w=W)
                nc.vector.tensor_max(v3, s4[:, :, 0, :], s4[:, :, 1, :])
                # Horizontal max of col pairs -> o[q*(W//2) + w'].
                o = opool.tile([P, OUT_F], F32, name=oname, tag=oname)
                vp = vm.rearrange("p (k two) -> p k two", two=2)
                nc.vector.tensor_max(o, vp[:, :, 0], vp[:, :, 1])
                nc.scalar.dma_start(outs[oname][i], o)

            def pe_combine(dst, c_t, a_t, a_eye, lo, hi):
                # dst[:, lo:hi] = c +/- a via identity matmuls into PSUM,
                # copied to SBUF by the (otherwise idle) scalar engine.
                for p0 in range(lo, hi, PS_F):
                    ps = pspool.tile([P, PS_F], F32, name="ps", tag="ps")
                    for k0 in range(0, PS_F, MM_F):
                        sl = slice(p0 + k0, p0 + k0 + MM_F)
                        psl = slice(k0, k0 + MM_F)
                        nc.tensor.matmul(
                            ps[:, psl], eye, c_t[:, sl], start=True, stop=False
                        )
                        nc.tensor.matmul(
                            ps[:, psl], a_eye, a_t[:, sl], start=False, stop=True
                        )
                    nc.scalar.copy(dst[:, p0 : p0 + PS_F], ps)

            for i in range(N_ITERS):
                c_t = inpool.tile([P, TILE_F], F32, name="c_t", tag="c_t")
                nc.sync.dma_start(c_t, ins["x_center"][i])
                a_t = inpool.tile([P, TILE_F], F32, name="a_t", tag="a_t")
                nc.sync.dma_start(a_t, ins["x_abs"][i])
                t_t = inpool.tile([P, TILE_F], F32, name="t_t", tag="t_t")
                nc.sync.dma_start(t_t, ins["x_true"][i])

                # sum = c + a entirely on PE; diff = c - a split PE/DVE.
                s = spool.tile([P, TILE_F], F32, name="s", tag="sd")
                pe_combine(s, c_t, a_t, eye, 0, TILE_F)
                d = spool.tile([P, TILE_F], F32, name="d", tag="sd")
                pe_combine(d, c_t, a_t, neye, 0, N_DIFF_PE)
                nc.vector.tensor_sub(
                    d[:, N_DIFF_PE:], c_t[:, N_DIFF_PE:], a_t[:, N_DIFF_PE:]
                )

                pool22(c_t, "out_c", i)
                pool22(d, "out_min", i)
                pool22(s, "out_max", i)
                pool22(t_t, "out_true", i)

    _split_excess_waits(nc)
    return nc


def _get_nc():
    if "nc" not in _CACHE:
        _CACHE["nc"] = _build_nc()
    return _CACHE["nc"]


def _shard_inputs(inputs):
    eye = np.eye(P, dtype=np.float32)
    idents = np.stack([eye, -eye])
    in_maps = []
    for k in range(N_CORES):
        sl = slice(k * CPC, (k + 1) * CPC)
        m = {
            nm: np.ascontiguousarray(inputs[nm][sl], dtype=np.float32).reshape(
                N_ITERS, P, TILE_F
            )
            for nm in IN_STREAMS
        }
        m["idents"] = idents
        in_maps.append(m)
    return in_maps


def _gather_outputs(results):
    outs = []
    for nm in OUT_STREAMS:
        outs.append(
            np.concatenate(
                [results[k][nm].reshape(CPC, H // 2, W // 2) for k in range(N_CORES)],
                axis=0,
            )
        )
    return tuple(outs)


def _run(inputs, **kwargs):
    nc = _get_nc()
    in_maps = _shard_inputs(inputs)
    return run_bass_kernel_spmd(nc, in_maps, core_ids=list(range(N_CORES)), **kwargs)


def kernel(x_center, x_abs, x_true):
    res = _run({"x_center": x_center, "x_abs": x_abs, "x_true": x_true})
    return _gather_outputs(res.results)


# revision 13
# speedup vs baseline: 1.1668x; 1.1369x over previous
"""Trainium2 Bass kernel for AbstractMaxpool2D.

Computes, for inputs x_center/x_abs/x_true of shape [128, 512, 512] f32:
  out_c    = maxpool2x2(x_center)
  out_min  = maxpool2x2(x_center - x_abs)
  out_max  = maxpool2x2(x_center + x_abs)
  out_true = maxpool2x2(x_true)
each [128, 256, 256] f32.  (The reference's relu-chain is exactly a 2x2
window max up to fp32 rounding; we compute the max directly.)

Sharding: channel dim C=128 split across 8 NeuronCores (16 channels each).
Per core the stream is a flat [8192, 512] row-major image; each tile is
[128 partitions x 4096] = 8 consecutive rows per partition, so both the
vertical (row-pair) and horizontal (col-pair) max reductions are
per-partition DVE ops and every DMA is a fully contiguous 2MB transfer.
"""

import numpy as np

try:
    import concourse.bass as bass
except ImportError:  # pragma: no cover - fallback for fresh grading dir
    import sys

    sys.path.insert(0, "/opt/trn_rl_repo")
    import concourse.bass as bass

import concourse.tile as tile
from concourse import mybir
from concourse.bass_utils import run_bass_kernel_spmd

F32 = mybir.dt.float32

N_CORES = 8
C, H, W = 128, 512, 512
CPC = C // N_CORES  # channels per core
P = 128  # SBUF partitions
ROWS_PER_PART = 8  # input image rows held by one partition per tile
TILE_F = ROWS_PER_PART * W  # 4096 floats per partition per input tile
OUT_F = (ROWS_PER_PART // 2) * (W // 2)  # 1024 floats per partition per out tile
N_ITERS = (CPC * H) // (P * ROWS_PER_PART)  # 8

IN_STREAMS = ("x_center", "x_abs", "x_true")
OUT_STREAMS = ("out_c", "out_min", "out_max", "out_true")

_CACHE = {}


def _split_excess_waits(nc):
    """Each 64B ISA instruction has ONE sync-wait slot (EventSemaphore: 2).

    Tile's sem assignment can attach several waits to one instruction;
    walrus then fails with 'Too many sync wait commands'.  Move the excess
    onto standalone EventSemaphore (wait-only) instructions placed just
    before, on the same engine — semantically identical, sequencer executes
    them in order.
    """
    n = 0
    for func in nc.m.functions:
        for blk in func.blocks:
            new_insts = []
            for inst in blk.instructions:
                si = inst.sync_info
                cap = 2 if isinstance(inst, mybir.InstEventSemaphore) else 1
                if si is not None and len(si.on_wait) > cap:
                    waits = list(si.on_wait)
                    keep, extra = waits[-cap:], waits[:-cap]
                    for w in extra:
                        n += 1
                        nop = mybir.InstEventSemaphore(
                            name=f"I-waitsplit-{n}", ins=[], outs=[]
                        )
                        nop.engine = inst.engine
                        nop.sync_info = mybir.SyncInfo(on_wait=[w], on_update=[])
                        new_insts.append(nop)
                    inst.sync_info = mybir.SyncInfo(
                        on_wait=keep, on_update=list(si.on_update)
                    )
                new_insts.append(inst)
            blk.instructions = new_insts
    return n


MM_F = 512  # fp32 matmul moving-operand max free dim
PS_F = 1024  # psum tile free dim (2 banks); 2 matmul chunks per tile
# Engine balance: PE fp32 identity-matmul costs ~1.74us per 512 cols
# (2 logical mm -> 4 HW passes), DVE tensor_sub ~0.6us per 512 cols.
# With DVE also running the 4 pooling chains (~13.6us/iter), putting all
# of sum + 1024 cols of diff on PE equalizes PE/DVE at ~17us/iter, both
# under the ~20.5us/iter DMA budget.
N_DIFF_PE = 1024  # first 1024 cols of (c-a) computed on PE, rest on DVE


def _build_nc():
    nc = bass.Bass(trn_type="TRN2")
    ins = {
        nm: nc.dram_tensor(nm, [N_ITERS, P, TILE_F], F32, kind="ExternalInput")
        for nm in IN_STREAMS
    }
    # idents[0] = I, idents[1] = -I (fp32 matmuls self-load weights, so
    # alternating weights costs nothing; identity matmul is bit-exact).
    ident_in = nc.dram_tensor("idents", [2, P, P], F32, kind="ExternalInput")
    outs = {
        nm: nc.dram_tensor(nm, [N_ITERS, P, OUT_F], F32, kind="ExternalOutput")
        for nm in OUT_STREAMS
    }

    with tile.TileContext(nc) as tc:
        with tc.tile_pool(name="const", bufs=1) as cpool, tc.tile_pool(
            name="io_in", bufs=2
        ) as inpool, tc.tile_pool(name="scratch", bufs=3) as spool, tc.tile_pool(
            name="vmpool", bufs=2
        ) as vmpool, tc.tile_pool(name="io_out", bufs=2) as opool, tc.tile_pool(
            name="psum", bufs=4, space="PSUM"
        ) as pspool:
            eye = cpool.tile([P, P], F32, name="eye")
            nc.sync.dma_start(eye, ident_in[0])
            neye = cpool.tile([P, P], F32, name="neye")
            nc.sync.dma_start(neye, ident_in[1])

            def pool22(src, oname, i):
                # src: AP [P, TILE_F]; rows r=0..7 per partition at offset r*W.
                # Vertical max of row pairs (2q, 2q+1) -> vm[q*W + w].
                vm = vmpool.tile([P, TILE_F // 2], F32, name="vm", tag="vm")
                s4 = src.rearrange("p (q two w) -> p q two w", two=2, w=W)
                v3 = vm.rearrange("p (q w) -> p q w", w=W)
                nc.vector.tensor_max(v3, s4[:, :, 0, :], s4[:, :, 1, :])
                # Horizontal max of col pairs -> o[q*(W//2) + w'].
                o = opool.tile([P, OUT_F], F32, name=oname, tag=oname)
                vp = vm.rearrange("p (k two) -> p k two", two=2)
                nc.vector.tensor_max(o, vp[:, :, 0], vp[:, :, 1])
                nc.scalar.dma_start(outs[oname][i], o)

            def pe_combine(dst, c_t, a_t, a_eye, lo, hi):
                # dst[:, lo:hi] = c +/- a via identity matmuls into PSUM,
                # copied to SBUF by the (otherwise idle) scalar engine.
                for p0 in range(lo, hi, PS_F):
                    ps = pspool.tile([P, PS_F], F32, name="ps", tag="ps")
                    for k0 in range(0, PS_F, MM_F):
                        sl = slice(p0 + k0, p0 + k0 + MM_F)
                        psl = slice(k0, k0 + MM_F)
                        nc.tensor.matmul(
                            ps[:, psl], eye, c_t[:, sl], start=True, stop=False
                        )
                        nc.tensor.matmul(
                            ps[:, psl], a_eye, a_t[:, sl], start=False, stop=True
                        )
                    nc.scalar.copy(dst[:, p0 : p0 + PS_F], ps)

            for i in range(N_ITERS):
                c_t = inpool.tile([P, TILE_F], F32, name="c_t", tag="c_t")
                nc.sync.dma_start(c_t, ins["x_center"][i])
                a_t = inpool.tile([P, TILE_F], F32, name="a_t", tag="a_t")
                nc.sync.dma_start(a_t, ins["x_abs"][i])
                t_t = inpool.tile([P, TILE_F], F32, name="t_t", tag="t_t")
                nc.sync.dma_start(t_t, ins["x_true"][i])

                # sum = c + a entirely on PE; diff = c - a split PE/DVE.
                s = spool.tile([P, TILE_F], F32, name="s", tag="sd")
                pe_combine(s, c_t, a_t, eye, 0, TILE_F)
                d = spool.tile([P, TILE_F], F32, name="d", tag="sd")
                pe_combine(d, c_t, a_t, neye, 0, N_DIFF_PE)
                nc.vector.tensor_sub(
                    d[:, N_DIFF_PE:], c_t[:, N_DIFF_PE:], a_t[:, N_DIFF_PE:]
                )

                pool22(c_t, "out_c", i)
                pool22(d, "out_min", i)
                pool22(s, "out_max", i)
                pool22(t_t, "out_true", i)

    _split_excess_waits(nc)
    return nc


def _get_nc():
    if "nc" not in _CACHE:
        _CACHE["nc"] = _build_nc()
    return _CACHE["nc"]


def _shard_inputs(inputs):
    eye = np.eye(P, dtype=np.float32)
    idents = np.stack([eye, -eye])
    in_maps = []
    for k in range(N_CORES):
        sl = slice(k * CPC, (k + 1) * CPC)
        m = {
            nm: np.ascontiguousarray(inputs[nm][sl], dtype=np.float32).reshape(
                N_ITERS, P, TILE_F
            )
            for nm in IN_STREAMS
        }
        m["idents"] = idents
        in_maps.append(m)
    return in_maps


def _gather_outputs(results):
    outs = []
    for nm in OUT_STREAMS:
        outs.append(
            np.concatenate(
                [results[k][nm].reshape(CPC, H // 2, W // 2) for k in range(N_CORES)],
                axis=0,
            )
        )
    return tuple(outs)


def _run(inputs, **kwargs):
    nc = _get_nc()
    in_maps = _shard_inputs(inputs)
    return run_bass_kernel_spmd(nc, in_maps, core_ids=list(range(N_CORES)), **kwargs)


def kernel(x_center, x_abs, x_true):
    res = _run({"x_center": x_center, "x_abs": x_abs, "x_true": x_true})
    return _gather_outputs(res.results)
